# revision 10
# baseline (speedup 1.0000x reference)
"""Correlation cost-volume kernel for Trainium2 (8 NeuronCores).

out[b,d,h,w] = sum_c left[b,c,h,w] * right[b,c,h,w-shift[d]]
  left/right: [4, 64, 256, 512] f32, shift: arange(96) -> out [4, 96, 256, 512] f32

Strategy:
  - Shard (b, h-half) across 8 cores: per-core left/right [64, 128, 512], no halo
    (shifts are along W only), no collectives.
  - Per (h, w-subtile of 32): the cost volume is a 96-wide anti-band of the
    Gram matrix G[i, j] = sum_c L[c, wg+i] * R[c, wg-95+j], computed as
    TensorEngine matmuls [K=64, M=32, N=127] in bf16 (PSUM accumulates f32).
  - Two h rows are packed in partitions 0-63 / 64-127 (row groups 0/64); four
    w-subtiles go to PSUM col-groups 0/32/64/96 via tile_position.  One PSUM
    BANK per (h-parity): the 4 w-chunks land at bank cols t*127 (sequential
    same-position matmuls into one bank are safe; only concurrent row-group
    matmuls into one bank fault).
  - Band output: one fat [128, 508] PSUM->SBUF bf16 copy per (pair, parity)
    (vector engine for par 0, scalar for par 1), then ONE DMA per 4 h-pairs
    straight into the output buffer with 8128-byte contiguous lines.
    Key layout trick: per w-row i the 127 gram cols land at flat offset
    127*(32g+i) + t*... i.e. each SBUF partition's cols are CONTIGUOUS in
    DRAM; the 96-wide anti-band is extracted on the host (j = p%32 + 95 - d).
    Small DMA lines are poison (~54 ns fixed cost per line at the DGE), so
    everything is shaped to keep lines >= 2 KB.
  - Host: pack/cast inputs to bf16, gather + upcast + transpose the output.
"""
import sys

sys.path.insert(0, "/opt/trn_rl_repo")

import numpy as np
import ml_dtypes

import concourse.bass as bass
import concourse.mybir as mybir
import concourse.tile as tile
from concourse.ap import AP
from concourse.bass_utils import run_bass_kernel_spmd
from concourse.vector_clock import ScopedClock

B, C, H, W, D = 4, 64, 256, 512, 96
HC = H // 2          # 128 h rows per core
T = 32               # w-subtile size (one PSUM col-group)
NT4 = 4              # w-chunks of 128 per h row
NG = T + D - 1       # 127 gram columns per subtile
BLK = 8              # h rows per block
NBLK = HC // BLK     # 8 blocks
QP = 2               # h-pairs fused per output DMA
PAIR_COLS = (D - 1) + W + W  # 95 pad + 512 R + 512 L = 1119
R_OFF = D - 1        # R data starts at col 95 within a pair's R region
L_OFF = (D - 1) + W  # L data starts at col 607
PROW = NT4 * NG      # psum/out cols per (pair, parity): 4*127 = 508
ROW = 2 * PROW       # out_sb cols per h-pair: (par, t, j) = 1016
GROW = QP * ROW      # out_sb cols per group of 4 pairs = 4064
NGRP = HC // (2 * QP)  # 16 output groups

BF16 = mybir.dt.bfloat16
F32 = mybir.dt.float32


_orig_add_instruction = tile.TileContext._add_instruction


def _patched_add_instruction(self, inst):
    # This walrus build allows at most ONE sync-wait per instruction: peel
    # extra waits onto single-wait NOPs on the same engine, just before it.
    si = inst.sync_info
    if si is not None and len(si.on_wait) > 1:
        waits = list(si.on_wait)
        for w in waits[:-1]:
            nop = mybir.InstNoOp(
                name=self.nc.get_next_instruction_name(),
                text_hint="split_wait",
                bass_nofuse=True,
            )
            nop.engine = inst.engine
            nop.sync_info = mybir.SyncInfo(on_wait=[w], on_update=[])
            _orig_add_instruction(self, nop)
        si.on_wait = waits[-1:]
    _orig_add_instruction(self, inst)


tile.TileContext._add_instruction = _patched_add_instruction


def _patched_drain_and_barrier(self, tick_clock, wait_clock):
    # This walrus build allows only ONE sync-wait on the tail Drain CTRL
    # instruction; split the final-clock waits across single-wait NOPs.
    nc = self.nc
    probe = nc.sync.nop(nofuse=True, hint="drain_waits")
    wait_clock.add_sem_waits(probe.ins, ScopedClock({None: tick_clock.global_clock}))
    waits = list(probe.ins.sync_info.on_wait)
    probe.ins.sync_info.on_wait = waits[:1]
    for w in waits[1:]:
        n = nc.sync.nop(nofuse=True, hint="drain_waits")
        n.ins.sync_info = mybir.SyncInfo(on_wait=[w], on_update=[])
    nc.sync.drain()
    nc.all_engine_barrier()
    assert self.sems is not None
    popped = nc._tile_sem_poison_stack.pop()
    assert popped is self._sem_poison
    nc.clear_and_free_semaphores(list(self.sems.allocated().values()))
    nc.all_engine_barrier()


tile.TileContext._drain_and_barrier = _patched_drain_and_barrier


def build_graph():
    nc = bass.Bass()
    lr_ext = nc.declare_dram_parameter("lrpack", [128, HC // 2, 2 * W], BF16, isOutput=False)
    out_ext = nc.declare_dram_parameter("out", [NGRP, 128, GROW], BF16, isOutput=True)

    with tile.TileContext(nc) as tc:
        with (
            tc.tile_pool(name="inp", bufs=6) as in_pool,
            tc.tile_pool(name="outsb", bufs=6) as out_pool,
            tc.tile_pool(name="psum", bufs=8, space="PSUM") as psum_pool,
        ):
            out_sb = None
            for blk in range(NBLK):
                # ---- load one block: 8 h-pairs -------------------------------
                blk_tile = in_pool.tile([128, (BLK // 2) * PAIR_COLS], BF16)
                # zero the 95-column left-pad of each pair's R region
                pad_ap = AP(
                    tensor=blk_tile.tensor,
                    offset=blk_tile.offset,
                    ap=[[blk_tile.tensor.shape[1], 128], [PAIR_COLS, BLK // 2], [1, R_OFF]],
                )
                nc.gpsimd.memset(pad_ap, 0.0)
                h2_0 = blk * (BLK // 2)
                # host packs R||L contiguously: one DMA, 2048-byte runs into
                # cols [R_OFF, R_OFF + 1024) = [95-col pad][512 R][512 L]
                # two half-loads per block: pair 0-1 matmuls only depend on
                # the first one (deps are AP-overlap based), so compute starts
                # after half the load
                for sl in range(2):
                    np_ = BLK // 4
                    src_rl = lr_ext[:, h2_0 + sl * np_ : h2_0 + (sl + 1) * np_, :]
                    dst_rl = AP(
                        tensor=blk_tile.tensor,
                        offset=blk_tile.offset + sl * np_ * PAIR_COLS + R_OFF,
                        ap=[[blk_tile.tensor.shape[1], 128], [PAIR_COLS, np_], [1, 2 * W]],
                    )
                    nc.sync.dma_start(dst_rl, src_rl)

                # ---- compute: per h-pair, 4 w-chunks x 4 col-groups ----------
                for j2 in range(BLK // 2):
                    base = j2 * PAIR_COLS
                    q = j2 % QP
                    if q == 0:
                        out_sb = out_pool.tile([128, GROW], BF16)
                    for par in range(2):
                        p0 = 64 * par
                        # one PSUM bank per parity; w-chunk t at bank cols t*127
                        ps = psum_pool.tile([128, PROW], F32)
                        for t in range(NT4):
                            w0 = t * 128
                            for g in range(4):
                                wg = w0 + T * g
                                lhsT = blk_tile[p0 : p0 + 64, base + L_OFF + wg : base + L_OFF + wg + T]
                                rhs = blk_tile[p0 : p0 + 64, base + wg : base + wg + NG]
                                nc.tensor.matmul(
                                    ps[32 * g : 32 * g + 32, t * NG : (t + 1) * NG],
                                    lhsT=lhsT,
                                    rhs=rhs,
                                    start=True,
                                    stop=True,
                                    tile_position=(p0, 32 * g),
                                )
                        # one fat PSUM->SBUF bf16 copy per (pair, parity)
                        dst = out_sb[:, q * ROW + par * PROW : q * ROW + (par + 1) * PROW]
                        if par == 0:
                            nc.vector.tensor_copy(dst, ps[:, 0:PROW])
                        else:
                            nc.scalar.copy(dst, ps[:, 0:PROW])
                    # one DMA per QP pairs: per-partition cols are contiguous
                    # in DRAM -> 8128-byte lines
                    if q == QP - 1:
                        grp = (blk * (BLK // 2) + j2) // QP
                        dst = AP(
                            tensor=out_ext,
                            offset=grp * 128 * GROW,
                            ap=[[GROW, 128], [1, GROW]],
                        )
                        # separate HWDGE ring (qAct) so in/out descriptor
                        # streams don't bubble each other
                        nc.scalar.dma_start(dst, out_sb[:])
    return nc


_CACHED = {}


def _get_graph():
    if "nc" not in _CACHED:
        _CACHED["nc"] = build_graph()
    return _CACHED["nc"]


def _pack_core(left_b, right_b, h0):
    """left_b/right_b: [C, H, W] f32 for one batch -> lrpack [128, 64, 1024] bf16.

    Layout: R row then L row contiguously (SBUF gets [pad|R|L] in one DMA);
    h-parity on partition halves (even h -> partitions 0-63, odd -> 64-127).
    """
    ls = left_b[:, h0 : h0 + HC, :]
    rs = right_b[:, h0 : h0 + HC, :]
    pack = np.empty((128, HC // 2, 2 * W), dtype=np.float32)
    pack[0:64, :, 0:W] = rs[:, 0::2, :]
    pack[64:128, :, 0:W] = rs[:, 1::2, :]
    pack[0:64, :, W : 2 * W] = ls[:, 0::2, :]
    pack[64:128, :, W : 2 * W] = ls[:, 1::2, :]
    return pack.astype(ml_dtypes.bfloat16)


def _run(inputs, trace=False):
    left = np.asarray(inputs["left"], dtype=np.float32)
    right = np.asarray(inputs["right"], dtype=np.float32)
    shift = np.asarray(inputs["shift"])

    nc = _get_graph()
    in_maps = []
    for core in range(8):
        b, half = core // 2, core % 2
        in_maps.append({"lrpack": _pack_core(left[b], right[b], half * HC)})

    res = run_bass_kernel_spmd(nc, in_maps, core_ids=list(range(8)), trace=trace)

    # band extraction: oc[grp, p, q, par, t, j] holds G for h = grp*8 + 2q + par,
    # w = 128t + p, at gram col j, with d = (p % 32) + 95 - j
    j_idx = (np.arange(128) % 32)[:, None] + 95 - np.arange(D)[None, :]  # [128, 96]
    j_idx_b = j_idx[None, :, None, None, None, :]
    out = np.empty((B, D, H, W), dtype=np.float32)
    for core in range(8):
        b, half = core // 2, core % 2
        oc = np.asarray(res.results[core]["out"]).reshape(NGRP, 128, QP, 2, NT4, NG)
        band = np.take_along_axis(oc, j_idx_b, axis=5)  # [16, 128, 4, 2, 4, 96]
        # -> [d, h=(grp,q,par), w=(t,p)]
        band = band.transpose(5, 0, 2, 3, 4, 1).reshape(D, HC, W).astype(np.float32)
        out[b, :, half * HC : (half + 1) * HC, :] = band

    # band covers integer shifts 0..95; remap if shift isn't exactly arange
    s = np.asarray(shift, dtype=np.float64)
    if not np.allclose(s, np.arange(D)):
        si = np.rint(s).astype(np.int64)
        if np.allclose(s, si) and si.min() >= 0 and si.max() < D:
            out = out[:, si, :, :]
        else:
            raise NotImplementedError(f"unsupported shift vector: {s}")
    return out, res


def kernel(**inputs) -> np.ndarray:
    out, _ = _run(inputs, trace=False)
    return out


# revision 11
# speedup vs baseline: 1.0195x; 1.0195x over previous
"""Correlation cost-volume kernel for Trainium2 (8 NeuronCores).

out[b,d,h,w] = sum_c left[b,c,h,w] * right[b,c,h,w-shift[d]]
  left/right: [4, 64, 256, 512] f32, shift: arange(96) -> out [4, 96, 256, 512] f32

Strategy:
  - Shard (b, h-half) across 8 cores: per-core left/right [64, 128, 512], no halo
    (shifts are along W only), no collectives.
  - Per (h, w-subtile of 32): the cost volume is a 96-wide anti-band of the
    Gram matrix G[i, j] = sum_c L[c, wg+i] * R[c, wg-95+j], computed as
    TensorEngine matmuls [K=64, M=32, N=127] in bf16 (PSUM accumulates f32).
  - Two h rows are packed in partitions 0-63 / 64-127 (row groups 0/64); four
    w-subtiles go to PSUM col-groups 0/32/64/96 via tile_position.  One PSUM
    BANK per (h-parity): the 4 w-chunks land at bank cols t*127 (sequential
    same-position matmuls into one bank are safe; only concurrent row-group
    matmuls into one bank fault).
  - Band output: one fat [128, 508] PSUM->SBUF bf16 copy per (pair, parity)
    (vector engine for par 0, scalar for par 1), then ONE DMA per 4 h-pairs
    straight into the output buffer with 8128-byte contiguous lines.
    Key layout trick: per w-row i the 127 gram cols land at flat offset
    127*(32g+i) + t*... i.e. each SBUF partition's cols are CONTIGUOUS in
    DRAM; the 96-wide anti-band is extracted on the host (j = p%32 + 95 - d).
    Small DMA lines are poison (~54 ns fixed cost per line at the DGE), so
    everything is shaped to keep lines >= 2 KB.
  - Host: pack/cast inputs to bf16, gather + upcast + transpose the output.
"""
import sys

sys.path.insert(0, "/opt/trn_rl_repo")

import numpy as np
import ml_dtypes

import concourse.bass as bass
import concourse.mybir as mybir
import concourse.tile as tile
from concourse.ap import AP
from concourse.bass_utils import run_bass_kernel_spmd
from concourse.vector_clock import ScopedClock

B, C, H, W, D = 4, 64, 256, 512, 96
HC = H // 2          # 128 h rows per core
T = 32               # w-subtile size (one PSUM col-group)
NT4 = 4              # w-chunks of 128 per h row
NG = T + D - 1       # 127 gram columns per subtile
BLK = 8              # h rows per block
NBLK = HC // BLK     # 8 blocks
QP = 2               # h-pairs fused per output DMA
PAIR_COLS = (D - 1) + W + W  # 95 pad + 512 R + 512 L = 1119
R_OFF = D - 1        # R data starts at col 95 within a pair's R region
L_OFF = (D - 1) + W  # L data starts at col 607
PROW = NT4 * NG      # psum/out cols per (pair, parity): 4*127 = 508
ROW = 2 * PROW       # out_sb cols per h-pair: (par, t, j) = 1016
GROW = QP * ROW      # out_sb cols per group of 4 pairs = 4064
NGRP = HC // (2 * QP)  # 16 output groups

BF16 = mybir.dt.bfloat16
F32 = mybir.dt.float32


_orig_add_instruction = tile.TileContext._add_instruction


def _patched_add_instruction(self, inst):
    # This walrus build allows at most ONE sync-wait per instruction: peel
    # extra waits onto single-wait NOPs on the same engine, just before it.
    si = inst.sync_info
    if si is not None and len(si.on_wait) > 1:
        waits = list(si.on_wait)
        for w in waits[:-1]:
            nop = mybir.InstNoOp(
                name=self.nc.get_next_instruction_name(),
                text_hint="split_wait",
                bass_nofuse=True,
            )
            nop.engine = inst.engine
            nop.sync_info = mybir.SyncInfo(on_wait=[w], on_update=[])
            _orig_add_instruction(self, nop)
        si.on_wait = waits[-1:]
    _orig_add_instruction(self, inst)


tile.TileContext._add_instruction = _patched_add_instruction


def _patched_drain_and_barrier(self, tick_clock, wait_clock):
    # This walrus build allows only ONE sync-wait on the tail Drain CTRL
    # instruction; split the final-clock waits across single-wait NOPs.
    nc = self.nc
    probe = nc.sync.nop(nofuse=True, hint="drain_waits")
    wait_clock.add_sem_waits(probe.ins, ScopedClock({None: tick_clock.global_clock}))
    waits = list(probe.ins.sync_info.on_wait)
    probe.ins.sync_info.on_wait = waits[:1]
    for w in waits[1:]:
        n = nc.sync.nop(nofuse=True, hint="drain_waits")
        n.ins.sync_info = mybir.SyncInfo(on_wait=[w], on_update=[])
    nc.sync.drain()
    nc.all_engine_barrier()
    assert self.sems is not None
    popped = nc._tile_sem_poison_stack.pop()
    assert popped is self._sem_poison
    nc.clear_and_free_semaphores(list(self.sems.allocated().values()))
    nc.all_engine_barrier()


tile.TileContext._drain_and_barrier = _patched_drain_and_barrier


def build_graph():
    nc = bass.Bass()
    lr_ext = nc.declare_dram_parameter("lrpack", [128, HC // 2, 2 * W], BF16, isOutput=False)
    out_ext = nc.declare_dram_parameter("out", [NGRP, 128, GROW], BF16, isOutput=True)

    with tile.TileContext(nc) as tc:
        with (
            tc.tile_pool(name="inp", bufs=6) as in_pool,
            tc.tile_pool(name="outsb", bufs=6) as out_pool,
            tc.tile_pool(name="psum", bufs=8, space="PSUM") as psum_pool,
        ):
            out_sb = None
            for blk in range(NBLK):
                # ---- load one block: 8 h-pairs -------------------------------
                blk_tile = in_pool.tile([128, (BLK // 2) * PAIR_COLS], BF16)
                # zero the 95-column left-pad of each pair's R region
                pad_ap = AP(
                    tensor=blk_tile.tensor,
                    offset=blk_tile.offset,
                    ap=[[blk_tile.tensor.shape[1], 128], [PAIR_COLS, BLK // 2], [1, R_OFF]],
                )
                nc.vector.memset(pad_ap, 0.0)
                h2_0 = blk * (BLK // 2)
                # host packs R||L contiguously: one DMA, 2048-byte runs into
                # cols [R_OFF, R_OFF + 1024) = [95-col pad][512 R][512 L]
                src_rl = lr_ext[:, h2_0 : h2_0 + BLK // 2, :]
                dst_rl = AP(
                    tensor=blk_tile.tensor,
                    offset=blk_tile.offset + R_OFF,
                    ap=[[blk_tile.tensor.shape[1], 128], [PAIR_COLS, BLK // 2], [1, 2 * W]],
                )
                nc.sync.dma_start(dst_rl, src_rl)

                # ---- compute: per h-pair, 4 w-chunks x 4 col-groups ----------
                for j2 in range(BLK // 2):
                    base = j2 * PAIR_COLS
                    q = j2 % QP
                    if q == 0:
                        out_sb = out_pool.tile([128, GROW], BF16)
                    for par in range(2):
                        p0 = 64 * par
                        # one PSUM bank per parity; w-chunk t at bank cols t*127
                        ps = psum_pool.tile([128, PROW], F32)
                        for t in range(NT4):
                            w0 = t * 128
                            for g in range(4):
                                wg = w0 + T * g
                                lhsT = blk_tile[p0 : p0 + 64, base + L_OFF + wg : base + L_OFF + wg + T]
                                rhs = blk_tile[p0 : p0 + 64, base + wg : base + wg + NG]
                                nc.tensor.matmul(
                                    ps[32 * g : 32 * g + 32, t * NG : (t + 1) * NG],
                                    lhsT=lhsT,
                                    rhs=rhs,
                                    start=True,
                                    stop=True,
                                    tile_position=(p0, 32 * g),
                                )
                        # one fat PSUM->SBUF bf16 copy per (pair, parity)
                        dst = out_sb[:, q * ROW + par * PROW : q * ROW + (par + 1) * PROW]
                        if par == 0:
                            nc.vector.tensor_copy(dst, ps[:, 0:PROW])
                        else:
                            nc.scalar.copy(dst, ps[:, 0:PROW])
                    # one DMA per QP pairs: per-partition cols are contiguous
                    # in DRAM -> 8128-byte lines
                    if q == QP - 1:
                        grp = (blk * (BLK // 2) + j2) // QP
                        dst = AP(
                            tensor=out_ext,
                            offset=grp * 128 * GROW,
                            ap=[[GROW, 128], [1, GROW]],
                        )
                        # separate HWDGE ring (qAct) so in/out descriptor
                        # streams don't bubble each other; alternate rings to
                        # spread dispatch cost
                        eng = nc.scalar if grp % 2 == 0 else nc.sync
                        eng.dma_start(dst, out_sb[:])
    return nc


_CACHED = {}


def _get_graph():
    if "nc" not in _CACHED:
        _CACHED["nc"] = build_graph()
    return _CACHED["nc"]


def _pack_core(left_b, right_b, h0):
    """left_b/right_b: [C, H, W] f32 for one batch -> lrpack [128, 64, 1024] bf16.

    Layout: R row then L row contiguously (SBUF gets [pad|R|L] in one DMA);
    h-parity on partition halves (even h -> partitions 0-63, odd -> 64-127).
    """
    ls = left_b[:, h0 : h0 + HC, :]
    rs = right_b[:, h0 : h0 + HC, :]
    pack = np.empty((128, HC // 2, 2 * W), dtype=np.float32)
    pack[0:64, :, 0:W] = rs[:, 0::2, :]
    pack[64:128, :, 0:W] = rs[:, 1::2, :]
    pack[0:64, :, W : 2 * W] = ls[:, 0::2, :]
    pack[64:128, :, W : 2 * W] = ls[:, 1::2, :]
    return pack.astype(ml_dtypes.bfloat16)


def _run(inputs, trace=False):
    left = np.asarray(inputs["left"], dtype=np.float32)
    right = np.asarray(inputs["right"], dtype=np.float32)
    shift = np.asarray(inputs["shift"])

    nc = _get_graph()
    in_maps = []
    for core in range(8):
        b, half = core // 2, core % 2
        in_maps.append({"lrpack": _pack_core(left[b], right[b], half * HC)})

    res = run_bass_kernel_spmd(nc, in_maps, core_ids=list(range(8)), trace=trace)

    # band extraction: oc[grp, p, q, par, t, j] holds G for h = grp*8 + 2q + par,
    # w = 128t + p, at gram col j, with d = (p % 32) + 95 - j
    j_idx = (np.arange(128) % 32)[:, None] + 95 - np.arange(D)[None, :]  # [128, 96]
    j_idx_b = j_idx[None, :, None, None, None, :]
    out = np.empty((B, D, H, W), dtype=np.float32)
    for core in range(8):
        b, half = core // 2, core % 2
        oc = np.asarray(res.results[core]["out"]).reshape(NGRP, 128, QP, 2, NT4, NG)
        band = np.take_along_axis(oc, j_idx_b, axis=5)  # [16, 128, 4, 2, 4, 96]
        # -> [d, h=(grp,q,par), w=(t,p)]
        band = band.transpose(5, 0, 2, 3, 4, 1).reshape(D, HC, W).astype(np.float32)
        out[b, :, half * HC : (half + 1) * HC, :] = band

    # band covers integer shifts 0..95; remap if shift isn't exactly arange
    s = np.asarray(shift, dtype=np.float64)
    if not np.allclose(s, np.arange(D)):
        si = np.rint(s).astype(np.int64)
        if np.allclose(s, si) and si.min() >= 0 and si.max() < D:
            out = out[:, si, :, :]
        else:
            raise NotImplementedError(f"unsupported shift vector: {s}")
    return out, res


def kernel(**inputs) -> np.ndarray:
    out, _ = _run(inputs, trace=False)
    return out
